# revision 3
# baseline (speedup 1.0000x reference)
"""Trainium2 Bass kernel for nn_AdaptiveSample (sparse adaptive 5x5 sampling).

Pixel-major rewrite: out[b,c,y,x] = sum_d softmax_d(valid*pos*guide) * f[b,c,y+dy,x+dx]

Sharding: H=256 over 8 cores (32 rows each, halos resolved on host).
Per-core layout: lane = (b, xq) with xq 64 x-blocks of 8 pixels; free = (y, c, xi).
  - softmax weights live in the same lane layout [128, D, y, xi], so the
    per-pixel weight multiplies features via a size-1-axis broadcast over c
    on the DVE directly (no partition broadcast machinery at all).
  - accumulation over the D offsets runs on the TensorEngine as identity
    matmuls accumulating in PSUM (start/stop groups), freeing the DVE of adds.
  - a few product passes run on the Pool engine (gpsimd) to balance DVE.
  - Activation engine does the exps and the 4 PSUM->SBUF evacuations (bf16).
Features are staged as two x-shifted slabs (even/odd dx) so every product AP
starts at an even bf16 element offset.
"""
import os
import sys

for _p in ("/opt/trn_rl_repo",):
    if os.path.isdir(_p) and _p not in sys.path:
        sys.path.append(_p)

import numpy as np
import ml_dtypes

from concourse import bass, mybir
from concourse import tile
from concourse.bass_utils import run_bass_kernel_spmd

BF16 = ml_dtypes.bfloat16
F32 = np.float32

B, C, H, W = 2, 32, 256, 512
KS, PAD, DMAX = 5, 2, 192.0
NCORES = 8
HS = H // NCORES          # 32 rows per core
NXQ = 64                  # x blocks per row
XB = W // NXQ             # 8 pixels per block
XHALO = XB + 2 * PAD      # 12 slab columns per block
YHALO = HS + 2 * PAD      # 36 slab rows
NPIX = HS * XB            # 256 pixel-plane elems per lane

_graph_cache = {}


def _runs(dyv, dxv):
    """Group the D unique offsets into runs of consecutive dx at equal dy.
    Offsets are sorted by (dy, dx) already (np.unique order)."""
    runs = []
    d0 = 0
    D = len(dyv)
    while d0 < D:
        d1 = d0 + 1
        while d1 < D and dyv[d1] == dyv[d0] and dxv[d1] == dxv[d1 - 1] + 1:
            d1 += 1
        runs.append((d0, d1, int(dyv[d0]), int(dxv[d0])))
        d0 = d1
    return runs


def _build_graph(D, dyv, dxv, pos_d, counts, niter=1, pool_units=6):
    """pool_units: how many (d, y-half) product passes run on the Pool engine
    instead of the DVE (Pool is ~3.7x slower per pass but otherwise idle)."""
    nc = bass.Bass(trn_type="TRN2", debug=False, enable_partition_id=False)
    dt_bf = mybir.dt.bfloat16
    dt_f32 = mybir.dt.float32

    sle_p = nc.declare_dram_parameter("sle", [128, YHALO, C, XHALO], dt_bf, isOutput=False)
    slo_p = nc.declare_dram_parameter("slo", [128, YHALO, C, XHALO], dt_bf, isOutput=False)
    g_p = nc.declare_dram_parameter("g", [128, D, HS, XB], dt_bf, isOutput=False)
    dpl_p = nc.declare_dram_parameter("dpl", [128, YHALO, XHALO], dt_bf, isOutput=False)
    id_p = nc.declare_dram_parameter("ident", [128, 128], dt_bf, isOutput=False)
    out_ext = nc.declare_dram_parameter("out", [128, HS, C, XB], dt_bf, isOutput=True)

    MULT = mybir.AluOpType.mult
    ADD = mybir.AluOpType.add
    IS_GT = mybir.AluOpType.is_gt
    IS_LT = mybir.AluOpType.is_lt
    EXP = mybir.ActivationFunctionType.Exp
    COPY = mybir.ActivationFunctionType.Copy

    runs = _runs(dyv, dxv)

    with tile.TileContext(nc) as tc:
        with (
            tc.tile_pool(name="big", bufs=1) as big,
            tc.tile_pool(name="pipe", bufs=1) as pipe,
            tc.tile_pool(name="prod", bufs=6) as prp,
            tc.tile_pool(name="prodp", bufs=4) as prpp,
            tc.tile_pool(name="ob", bufs=2) as obp,
            tc.tile_pool(name="psum", bufs=1, space="PSUM") as psp,
        ):
            g = pipe.tile([128, D, HS, XB], dt_bf, tag="g")
            dpl = pipe.tile([128, YHALO, XHALO], dt_bf, tag="dpl")
            ident = big.tile([128, 128], dt_bf, tag="ident")
            nc.sync.dma_start(out=dpl[:, :, :], in_=dpl_p[:, :, :])
            nc.sync.dma_start(out=g[:, :, :, :], in_=g_p[:, :, :, :])
            nc.sync.dma_start(out=ident[:, :], in_=id_p[:, :])
            sle = big.tile([128, YHALO, C, XHALO], dt_bf, tag="sle")
            slo = big.tile([128, YHALO, C, XHALO], dt_bf, tag="slo")
            # stream the y-halves used first ahead of the rest
            nc.sync.dma_start(out=sle[:, :20, :, :], in_=sle_p[:, :20, :, :])
            nc.sync.dma_start(out=slo[:, :20, :, :], in_=slo_p[:, :20, :, :])
            nc.sync.dma_start(out=sle[:, 20:, :, :], in_=sle_p[:, 20:, :, :])
            nc.sync.dma_start(out=slo[:, 20:, :, :], in_=slo_p[:, 20:, :, :])
            slabs = {0: sle, 1: slo}

            for _iter in range(niter):
                bias_vals = sorted({float(np.log(c_)) for c_ in counts})
                bias_tiles = {}
                for bv in bias_vals:
                    bt = pipe.tile([128, 1], dt_f32, tag=f"bias{bv:.4f}")
                    nc.vector.memset(bt[:, :], bv)
                    bias_tiles[bv] = bt

                # validity plane: v = (dpl > 0) * (dpl < DMAX)  [128, 36, 12]
                t1 = pipe.tile([128, YHALO, XHALO], dt_bf, tag="t1")
                t2 = pipe.tile([128, YHALO, XHALO], dt_bf, tag="t2")
                v = pipe.tile([128, YHALO, XHALO], dt_bf, tag="v")
                nc.vector.tensor_scalar(out=t1[:, :, :], in0=dpl[:, :, :],
                                        scalar1=0.0, scalar2=None, op0=IS_GT)
                nc.vector.tensor_scalar(out=t2[:, :, :], in0=dpl[:, :, :],
                                        scalar1=DMAX, scalar2=None, op0=IS_LT)
                nc.vector.tensor_tensor(v[:, :, :], t1[:, :, :], t2[:, :, :], MULT)

                # vg[d] = g[d] * v[dy_d:dy_d+HS, dx_d:dx_d+XB], runs share one op
                vg = pipe.tile([128, D, HS, XB], dt_bf, tag="vg")
                for (d0, d1, dy, dx0) in runs:
                    n = d1 - d0
                    vb = v[:, dy:dy + HS, dx0:dx0 + XB]
                    vsl = bass.AP(vb.tensor, vb.offset,
                                  [list(vb.ap[0]), [1, n], [XHALO, HS], [1, XB]])
                    nc.vector.tensor_tensor(vg[:, d0:d1, :, :], g[:, d0:d1, :, :],
                                            vsl, MULT)

                # e[d] = count_d * exp(pos_d * vg_d); extra c axis of size 1
                # so slices broadcast over channels later
                e = pipe.tile([128, D, HS, 1, XB], dt_bf, tag="e")
                for d in range(D):
                    bv = float(np.log(counts[d]))
                    nc.scalar.activation(e[:, d, :, 0, :], vg[:, d, :, :], EXP,
                                         bias=bias_tiles[bv][:, :], scale=float(pos_d[d]))

                # denominator tree-fold then reciprocal
                m = D // 2
                num_f = pipe.tile([128, m, HS, XB], dt_bf, tag="numf")
                nc.vector.tensor_tensor(num_f[:, :, :, :], e[:, 0:m, :, 0, :],
                                        e[:, m:2 * m, :, 0, :], ADD)
                lvl = num_f[:, :, :, :]
                n = m
                extra = [e[:, 2 * m + i, :, 0, :] for i in range(D - 2 * m)]
                li = 0
                while n > 1:
                    h2 = n // 2
                    nt = pipe.tile([128, h2, HS, XB], dt_bf, tag=f"nf{li}")
                    nc.vector.tensor_tensor(nt[:, :, :, :], lvl[:, 0:h2, :, :],
                                            lvl[:, h2:2 * h2, :, :], ADD)
                    if n % 2:
                        extra.append(lvl[:, 2 * h2, :, :])
                    lvl = nt[:, :, :, :]
                    n = h2
                    li += 1
                cur = lvl[:, 0, :, :]
                for i, ex in enumerate(extra):
                    dent = pipe.tile([128, HS, XB], dt_bf, tag=f"dx{i}")
                    nc.vector.tensor_tensor(dent[:, :, :], cur, ex, ADD)
                    cur = dent[:, :, :]
                rden_f = pipe.tile([128, HS, XB], dt_f32, tag="rdenf")
                nc.vector.reciprocal(rden_f[:, :, :], cur)
                rden = pipe.tile([128, 1, HS, XB], dt_bf, tag="rden")
                nc.vector.tensor_copy(rden[:, 0, :, :], rden_f[:, :, :])

                # normalized weights w = e * rden  [128, D, HS, 1, XB]
                wq = pipe.tile([128, D, HS, 1, XB], dt_bf, tag="wq")
                rb, _ = bass.broadcast_tensor_aps(rden[:, :, :, :], e[:, :, :, 0, :])
                nc.vector.tensor_tensor(wq[:, :, :, 0, :], e[:, :, :, 0, :], rb, MULT)

                # products + PE identity accumulation, per y-half.
                # pool_units product passes run on the Pool engine; spread the
                # chosen d's away from each other and across halves.
                spread = [d for d in (3, 9, 6, 1, 11, 5, 7, 0, 10, 2, 12, 4, 8) if d < D]
                pool_pairs = set()
                for i in range(min(pool_units, 2 * D)):
                    pool_pairs.add((spread[i % len(spread)], (i + i // len(spread)) % 2))
                for h in range(2):
                    y0 = 16 * h
                    pool_list = [d for d in range(D) if (d, h) in pool_pairs]
                    dve_list = [d for d in range(D) if (d, h) not in pool_pairs]
                    prods = {}
                    # Pool-engine products issued first (slow engine, runs in
                    # parallel with the DVE stream); accumulated last on PE.
                    for d in pool_list + dve_list:
                        dy, dx = int(dyv[d]), int(dxv[d])
                        src = slabs[dx % 2]
                        # slab xi base: even slab xi==dx, odd slab xi==dx-1
                        xs = dx if dx % 2 == 0 else dx - 1
                        f_ap = src[:, dy + y0:dy + y0 + 16, :, xs:xs + XB]
                        w_ap, _ = bass.broadcast_tensor_aps(
                            wq[:, d, y0:y0 + 16, :, :], f_ap)
                        if d in pool_list:
                            pr = prpp.tile([128, 16, C, XB], dt_bf, tag="prp")
                            nc.gpsimd.tensor_tensor(pr[:, :, :, :], f_ap, w_ap, MULT)
                        else:
                            pr = prp.tile([128, 16, C, XB], dt_bf, tag="pr")
                            nc.vector.tensor_tensor(pr[:, :, :, :], f_ap, w_ap, MULT)
                        prods[d] = pr
                    pss = [psp.tile([128, 2048], dt_f32, tag=f"q{q}", name=f"ps_{h}_{q}")
                           for q in range(2)]
                    mm_order = dve_list + pool_list
                    for di, d in enumerate(mm_order):
                        pv = prods[d][:, :, :, :].rearrange("p y c xi -> p (y c xi)")
                        for q in range(2):
                            for ci in range(4):
                                c0 = q * 2048 + ci * 512
                                nc.tensor.matmul(
                                    pss[q][:, ci * 512:(ci + 1) * 512],
                                    lhsT=ident[:, :], rhs=pv[:, c0:c0 + 512],
                                    start=(di == 0), stop=(di == D - 1))
                    for q in range(2):
                        ob = obp.tile([128, 8, C, XB], dt_bf, tag="ob")
                        nc.scalar.activation(
                            ob[:, :, :, :],
                            pss[q][:, :].rearrange("p (y c xi) -> p y c xi",
                                                   y=8, c=C, xi=XB), COPY)
                        nc.sync.dma_start(
                            out=out_ext[:, y0 + 8 * q:y0 + 8 * q + 8, :, :],
                            in_=ob[:, :, :, :])

    _split_excess_waits(nc)
    return nc


def _split_excess_waits(nc, max_waits=1):
    """walrus in this container rejects >1 chained sync-wait per instruction;
    spill extras onto preceding sequencer NOPs."""
    n = 0
    for fn in nc.m.functions:
        for bb in fn.blocks:
            new = []
            for inst in bb.instructions:
                si = inst.sync_info
                w = list(si.on_wait) if si is not None else []
                if len(w) > max_waits:
                    excess = w[max_waits:]
                    si.on_wait = w[:max_waits]
                    for i in range(0, len(excess), max_waits):
                        nop = mybir.InstNoOp(name=nc.get_next_instruction_name(), ins=[], outs=[])
                        nop.engine = inst.engine
                        nsi = nop.sync_info
                        if nsi is None:
                            nop.sync_info = mybir.SyncInfo(on_wait=excess[i:i + max_waits], on_update=[])
                        else:
                            nsi.on_wait = excess[i:i + max_waits]
                        nc.register_instruction(nop)
                        new.append(nop)
                        n += 1
                new.append(inst)
            bb.instructions = new
    return n


def _prep_inputs(depth, features, guide_weight, sample_idx):
    """Shard + lay out the full inputs for the 8 cores. Returns in_maps, meta."""
    si = np.asarray(sample_idx).astype(np.int64)
    vals, counts = np.unique(si, return_counts=True)
    D = len(vals)
    ctr = KS // 2
    px = (si % KS).astype(np.float64)
    py = (si // KS).astype(np.float64)
    Z = np.exp(-0.5 * np.sqrt((px - ctr) ** 2 + (py - ctr) ** 2)).sum()
    pos_d = np.exp(-0.5 * np.sqrt(((vals % KS) - ctr) ** 2 + ((vals // KS) - ctr) ** 2)) / Z
    dyv = (vals // KS).astype(int)          # 0..4 offsets in padded coords
    dxv = (vals % KS).astype(int)

    feats_bf = features.astype(BF16)
    # padded planes: y pad 2 each side; x pad 2 left, 3 right (odd slab shift)
    fpad = np.zeros((B, C, H + 4, W + 5), BF16)
    fpad[:, :, 2:2 + H, 2:2 + W] = feats_bf
    dpad = np.zeros((B, H + 4, W + 5), BF16)
    dpad[:, 2:2 + H, 2:2 + W] = depth.reshape(B, H, W).astype(BF16)

    swv = np.lib.stride_tricks.sliding_window_view  # read-only views
    in_maps = []
    ident = np.eye(128, dtype=BF16)
    gw = np.asarray(guide_weight)
    for core in range(NCORES):
        r0 = core * HS
        fr = fpad[:, :, r0:r0 + YHALO, :]                      # [B,C,36,517]
        win = swv(fr, XHALO, axis=3)                           # [B,C,36,506,12]
        sle = np.ascontiguousarray(
            win[:, :, :, 0:W:XB, :].transpose(0, 3, 2, 1, 4)).reshape(
            128, YHALO, C, XHALO)
        slo = np.ascontiguousarray(
            win[:, :, :, 1:W + 1:XB, :].transpose(0, 3, 2, 1, 4)).reshape(
            128, YHALO, C, XHALO)
        dr = dpad[:, r0:r0 + YHALO, :]                         # [B,36,517]
        dwin = swv(dr, XHALO, axis=2)                          # [B,36,506,12]
        dpl = np.ascontiguousarray(
            dwin[:, :, 0:W:XB, :].transpose(0, 2, 1, 3)).reshape(128, YHALO, XHALO)
        gsel = gw[:, r0:r0 + HS, :, :][..., vals]              # [B,HS,512,D]
        g = np.ascontiguousarray(
            gsel.reshape(B, HS, NXQ, XB, D).transpose(0, 2, 4, 1, 3)).reshape(
            128, D, HS, XB).astype(BF16)
        in_maps.append({"sle": sle, "slo": slo, "g": g, "dpl": dpl, "ident": ident})
    return in_maps, (D, dyv, dxv, pos_d, counts)


def kernel(depth, features, guide_weight, sample_idx):
    depth = np.asarray(depth)
    features = np.asarray(features)
    guide_weight = np.asarray(guide_weight)
    sample_idx = np.asarray(sample_idx)

    in_maps, meta = _prep_inputs(depth, features, guide_weight, sample_idx)
    D, dyv, dxv, pos_d, counts = meta

    key = (tuple(dyv), tuple(dxv), tuple(np.round(pos_d, 10)), tuple(counts))
    nc = _graph_cache.get(key)
    if nc is None:
        nc = _build_graph(D, dyv, dxv, pos_d, counts)
        _graph_cache[key] = nc

    res = run_bass_kernel_spmd(nc, in_maps, core_ids=list(range(NCORES)))

    out = np.empty((B, C, H, W), F32)
    for core in range(NCORES):
        r0 = core * HS
        o = res.results[core]["out"].astype(F32).reshape(B, NXQ, HS, C, XB)
        out[:, :, r0:r0 + HS, :] = o.transpose(0, 3, 2, 1, 4).reshape(B, C, HS, W)
    return out, features


# revision 5
# speedup vs baseline: 1.2007x; 1.2007x over previous
"""Trainium2 Bass kernel for nn_AdaptiveSample (sparse adaptive 5x5 sampling).

Pixel-major rewrite: out[b,c,y,x] = sum_d softmax_d(valid*pos*guide) * f[b,c,y+dy,x+dx]

Sharding: H=256 over 8 cores (32 rows each, halos resolved on host).
Per-core layout: lane = (b, xq) with xq 64 x-blocks of 8 pixels; free = (y, c, xi).
  - softmax weights live in the same lane layout [128, D, y, xi], so the
    per-pixel weight multiplies features via a size-1-axis broadcast over c
    on the DVE directly (no partition broadcast machinery at all).
  - accumulation over the D offsets runs on the TensorEngine as identity
    matmuls accumulating in PSUM (start/stop groups), freeing the DVE of adds.
  - a few product passes run on the Pool engine (gpsimd) to balance DVE.
  - Activation engine does the exps and the 4 PSUM->SBUF evacuations (bf16).
Features are staged as two x-shifted slabs (even/odd dx) so every product AP
starts at an even bf16 element offset.
"""
import os
import sys

for _p in ("/opt/trn_rl_repo",):
    if os.path.isdir(_p) and _p not in sys.path:
        sys.path.append(_p)

import numpy as np
import ml_dtypes

from concourse import bass, mybir
from concourse import tile
from concourse.bass_utils import run_bass_kernel_spmd

BF16 = ml_dtypes.bfloat16
F32 = np.float32

B, C, H, W = 2, 32, 256, 512
KS, PAD, DMAX = 5, 2, 192.0
NCORES = 8
HS = H // NCORES          # 32 rows per core
NXQ = 64                  # x blocks per row
XB = W // NXQ             # 8 pixels per block
XHALO = XB + 2 * PAD      # 12 slab columns per block
YHALO = HS + 2 * PAD      # 36 slab rows
NPIX = HS * XB            # 256 pixel-plane elems per lane

_graph_cache = {}


def _runs(dyv, dxv):
    """Group the D unique offsets into runs of consecutive dx at equal dy.
    Offsets are sorted by (dy, dx) already (np.unique order)."""
    runs = []
    d0 = 0
    D = len(dyv)
    while d0 < D:
        d1 = d0 + 1
        while d1 < D and dyv[d1] == dyv[d0] and dxv[d1] == dxv[d1 - 1] + 1:
            d1 += 1
        runs.append((d0, d1, int(dyv[d0]), int(dxv[d0])))
        d0 = d1
    return runs


def _build_graph(D, dyv, dxv, pos_d, counts, niter=1, pool_units=6):
    """pool_units: how many (d, y-half) product passes run on the Pool engine
    instead of the DVE (Pool is ~3.7x slower per pass but otherwise idle)."""
    nc = bass.Bass(trn_type="TRN2", debug=False, enable_partition_id=False)
    dt_bf = mybir.dt.bfloat16
    dt_f32 = mybir.dt.float32

    sle_p = nc.declare_dram_parameter("sle", [128, YHALO, C, XHALO], dt_bf, isOutput=False)
    slo_p = nc.declare_dram_parameter("slo", [128, YHALO, C, XHALO], dt_bf, isOutput=False)
    g_p = nc.declare_dram_parameter("g", [128, D, HS, XB], dt_bf, isOutput=False)
    dpl_p = nc.declare_dram_parameter("dpl", [128, YHALO, XHALO], dt_bf, isOutput=False)
    id_p = nc.declare_dram_parameter("ident", [128, 128], dt_bf, isOutput=False)
    out_ext = nc.declare_dram_parameter("out", [128, HS, C, XB], dt_bf, isOutput=True)

    MULT = mybir.AluOpType.mult
    ADD = mybir.AluOpType.add
    IS_GT = mybir.AluOpType.is_gt
    IS_LT = mybir.AluOpType.is_lt
    EXP = mybir.ActivationFunctionType.Exp
    COPY = mybir.ActivationFunctionType.Copy

    runs = _runs(dyv, dxv)

    with tile.TileContext(nc) as tc:
        with (
            tc.tile_pool(name="big", bufs=1) as big,
            tc.tile_pool(name="pipe", bufs=1) as pipe,
            tc.tile_pool(name="prod", bufs=6) as prp,
            tc.tile_pool(name="prodp", bufs=4) as prpp,
            tc.tile_pool(name="ob", bufs=2) as obp,
            tc.tile_pool(name="psum", bufs=1, space="PSUM") as psp,
        ):
            g = pipe.tile([128, D, HS, XB], dt_bf, tag="g")
            dpl = pipe.tile([128, YHALO, XHALO], dt_bf, tag="dpl")
            ident = big.tile([128, 128], dt_bf, tag="ident")
            nc.sync.dma_start(out=dpl[:, :, :], in_=dpl_p[:, :, :])
            nc.sync.dma_start(out=g[:, :, :, :], in_=g_p[:, :, :, :])
            nc.sync.dma_start(out=ident[:, :], in_=id_p[:, :])
            sle = big.tile([128, YHALO, C, XHALO], dt_bf, tag="sle")
            slo = big.tile([128, YHALO, C, XHALO], dt_bf, tag="slo")
            # stream the y-halves used first ahead of the rest
            nc.sync.dma_start(out=sle[:, :20, :, :], in_=sle_p[:, :20, :, :])
            nc.sync.dma_start(out=slo[:, :20, :, :], in_=slo_p[:, :20, :, :])
            nc.sync.dma_start(out=sle[:, 20:, :, :], in_=sle_p[:, 20:, :, :])
            nc.sync.dma_start(out=slo[:, 20:, :, :], in_=slo_p[:, 20:, :, :])
            slabs = {0: sle, 1: slo}

            bias_vals = sorted({float(np.log(c_)) for c_ in counts})
            bias_tiles = {}
            for bv in bias_vals:
                bt = pipe.tile([128, 1], dt_f32, tag=f"bias{bv:.4f}")
                nc.vector.memset(bt[:, :], bv)
                bias_tiles[bv] = bt

            for _iter in range(niter):

                # validity plane: v = (dpl > 0) * (dpl < DMAX)  [128, 36, 12]
                t1 = pipe.tile([128, YHALO, XHALO], dt_bf, tag="t1")
                t2 = pipe.tile([128, YHALO, XHALO], dt_bf, tag="t2")
                v = pipe.tile([128, YHALO, XHALO], dt_bf, tag="v")
                nc.vector.tensor_scalar(out=t1[:, :, :], in0=dpl[:, :, :],
                                        scalar1=0.0, scalar2=None, op0=IS_GT)
                nc.vector.tensor_scalar(out=t2[:, :, :], in0=dpl[:, :, :],
                                        scalar1=DMAX, scalar2=None, op0=IS_LT)
                nc.vector.tensor_tensor(v[:, :, :], t1[:, :, :], t2[:, :, :], MULT)

                # vg[d] = g[d] * v[dy_d:dy_d+HS, dx_d:dx_d+XB], runs share one op
                vg = pipe.tile([128, D, HS, XB], dt_bf, tag="vg")
                for (d0, d1, dy, dx0) in runs:
                    n = d1 - d0
                    vb = v[:, dy:dy + HS, dx0:dx0 + XB]
                    vsl = bass.AP(vb.tensor, vb.offset,
                                  [list(vb.ap[0]), [1, n], [XHALO, HS], [1, XB]])
                    nc.vector.tensor_tensor(vg[:, d0:d1, :, :], g[:, d0:d1, :, :],
                                            vsl, MULT)

                # e[d] = count_d * exp(pos_d * vg_d); extra c axis of size 1
                # so slices broadcast over channels later
                e = pipe.tile([128, D, HS, 1, XB], dt_bf, tag="e")
                for d in range(D):
                    bv = float(np.log(counts[d]))
                    nc.scalar.activation(e[:, d, :, 0, :], vg[:, d, :, :], EXP,
                                         bias=bias_tiles[bv][:, :], scale=float(pos_d[d]))

                # denominator tree-fold then reciprocal
                m = D // 2
                num_f = pipe.tile([128, m, HS, XB], dt_bf, tag="numf")
                nc.vector.tensor_tensor(num_f[:, :, :, :], e[:, 0:m, :, 0, :],
                                        e[:, m:2 * m, :, 0, :], ADD)
                lvl = num_f[:, :, :, :]
                n = m
                extra = [e[:, 2 * m + i, :, 0, :] for i in range(D - 2 * m)]
                li = 0
                while n > 1:
                    h2 = n // 2
                    nt = pipe.tile([128, h2, HS, XB], dt_bf, tag=f"nf{li}")
                    nc.vector.tensor_tensor(nt[:, :, :, :], lvl[:, 0:h2, :, :],
                                            lvl[:, h2:2 * h2, :, :], ADD)
                    if n % 2:
                        extra.append(lvl[:, 2 * h2, :, :])
                    lvl = nt[:, :, :, :]
                    n = h2
                    li += 1
                cur = lvl[:, 0, :, :]
                for i, ex in enumerate(extra):
                    dent = pipe.tile([128, HS, XB], dt_bf, tag=f"dx{i}")
                    nc.vector.tensor_tensor(dent[:, :, :], cur, ex, ADD)
                    cur = dent[:, :, :]
                rden_f = pipe.tile([128, HS, XB], dt_f32, tag="rdenf")
                nc.vector.reciprocal(rden_f[:, :, :], cur)
                rden = pipe.tile([128, 1, HS, XB], dt_bf, tag="rden")
                nc.vector.tensor_copy(rden[:, 0, :, :], rden_f[:, :, :])

                # normalized weights w = e * rden  [128, D, HS, 1, XB]
                wq = pipe.tile([128, D, HS, 1, XB], dt_bf, tag="wq")
                rb, _ = bass.broadcast_tensor_aps(rden[:, :, :, :], e[:, :, :, 0, :])
                nc.vector.tensor_tensor(wq[:, :, :, 0, :], e[:, :, :, 0, :], rb, MULT)

                # products + PE identity accumulation, per y-half.
                # pool_units product passes run on the Pool engine; spread the
                # chosen d's away from each other and across halves.
                spread = [d for d in (3, 9, 6, 1, 11, 5, 7, 0, 10, 2, 12, 4, 8) if d < D]
                pool_pairs = set()
                for i in range(min(pool_units, 2 * D)):
                    pool_pairs.add((spread[i % len(spread)], (i + i // len(spread)) % 2))
                for h in range(2):
                    y0 = 16 * h
                    pool_list = [d for d in range(D) if (d, h) in pool_pairs]
                    dve_list = [d for d in range(D) if (d, h) not in pool_pairs]
                    prods = {}
                    # Pool-engine products issued first (slow engine, runs in
                    # parallel with the DVE stream); accumulated last on PE.
                    for d in pool_list + dve_list:
                        dy, dx = int(dyv[d]), int(dxv[d])
                        src = slabs[dx % 2]
                        # slab xi base: even slab xi==dx, odd slab xi==dx-1
                        xs = dx if dx % 2 == 0 else dx - 1
                        f_ap = src[:, dy + y0:dy + y0 + 16, :, xs:xs + XB]
                        w_ap, _ = bass.broadcast_tensor_aps(
                            wq[:, d, y0:y0 + 16, :, :], f_ap)
                        if d in pool_list:
                            pr = prpp.tile([128, 16, C, XB], dt_bf, tag="prp")
                            nc.gpsimd.tensor_tensor(pr[:, :, :, :], f_ap, w_ap, MULT)
                        else:
                            pr = prp.tile([128, 16, C, XB], dt_bf, tag="pr")
                            nc.vector.tensor_tensor(pr[:, :, :, :], f_ap, w_ap, MULT)
                        prods[d] = pr
                    pss = [psp.tile([128, 2048], dt_f32, tag=f"q{q}", name=f"ps_{h}_{q}")
                           for q in range(2)]
                    mm_order = dve_list + pool_list
                    for di, d in enumerate(mm_order):
                        pv = prods[d][:, :, :, :].rearrange("p y c xi -> p (y c xi)")
                        for q in range(2):
                            for ci in range(4):
                                c0 = q * 2048 + ci * 512
                                nc.tensor.matmul(
                                    pss[q][:, ci * 512:(ci + 1) * 512],
                                    lhsT=ident[:, :], rhs=pv[:, c0:c0 + 512],
                                    start=(di == 0), stop=(di == D - 1))
                    for q in range(2):
                        ob = obp.tile([128, 8, C, XB], dt_bf, tag="ob")
                        nc.scalar.activation(
                            ob[:, :, :, :],
                            pss[q][:, :].rearrange("p (y c xi) -> p y c xi",
                                                   y=8, c=C, xi=XB), COPY)
                        nc.sync.dma_start(
                            out=out_ext[:, y0 + 8 * q:y0 + 8 * q + 8, :, :],
                            in_=ob[:, :, :, :])

    _split_excess_waits(nc)
    return nc


def _split_excess_waits(nc, max_waits=1):
    """walrus in this container rejects >1 chained sync-wait per instruction;
    spill extras onto preceding sequencer NOPs."""
    n = 0
    for fn in nc.m.functions:
        for bb in fn.blocks:
            new = []
            for inst in bb.instructions:
                si = inst.sync_info
                w = list(si.on_wait) if si is not None else []
                if len(w) > max_waits:
                    excess = w[max_waits:]
                    si.on_wait = w[:max_waits]
                    for i in range(0, len(excess), max_waits):
                        nop = mybir.InstNoOp(name=nc.get_next_instruction_name(), ins=[], outs=[])
                        nop.engine = inst.engine
                        nsi = nop.sync_info
                        if nsi is None:
                            nop.sync_info = mybir.SyncInfo(on_wait=excess[i:i + max_waits], on_update=[])
                        else:
                            nsi.on_wait = excess[i:i + max_waits]
                        nc.register_instruction(nop)
                        new.append(nop)
                        n += 1
                new.append(inst)
            bb.instructions = new
    return n


def _prep_inputs(depth, features, guide_weight, sample_idx):
    """Shard + lay out the full inputs for the 8 cores. Returns in_maps, meta."""
    si = np.asarray(sample_idx).astype(np.int64)
    vals, counts = np.unique(si, return_counts=True)
    D = len(vals)
    ctr = KS // 2
    px = (si % KS).astype(np.float64)
    py = (si // KS).astype(np.float64)
    Z = np.exp(-0.5 * np.sqrt((px - ctr) ** 2 + (py - ctr) ** 2)).sum()
    pos_d = np.exp(-0.5 * np.sqrt(((vals % KS) - ctr) ** 2 + ((vals // KS) - ctr) ** 2)) / Z
    dyv = (vals // KS).astype(int)          # 0..4 offsets in padded coords
    dxv = (vals % KS).astype(int)

    feats_bf = features.astype(BF16)
    # padded planes: y pad 2 each side; x pad 2 left, 3 right (odd slab shift)
    fpad = np.zeros((B, C, H + 4, W + 5), BF16)
    fpad[:, :, 2:2 + H, 2:2 + W] = feats_bf
    dpad = np.zeros((B, H + 4, W + 5), BF16)
    dpad[:, 2:2 + H, 2:2 + W] = depth.reshape(B, H, W).astype(BF16)

    swv = np.lib.stride_tricks.sliding_window_view  # read-only views
    in_maps = []
    ident = np.eye(128, dtype=BF16)
    gw = np.asarray(guide_weight)
    for core in range(NCORES):
        r0 = core * HS
        fr = fpad[:, :, r0:r0 + YHALO, :]                      # [B,C,36,517]
        win = swv(fr, XHALO, axis=3)                           # [B,C,36,506,12]
        sle = np.ascontiguousarray(
            win[:, :, :, 0:W:XB, :].transpose(0, 3, 2, 1, 4)).reshape(
            128, YHALO, C, XHALO)
        slo = np.ascontiguousarray(
            win[:, :, :, 1:W + 1:XB, :].transpose(0, 3, 2, 1, 4)).reshape(
            128, YHALO, C, XHALO)
        dr = dpad[:, r0:r0 + YHALO, :]                         # [B,36,517]
        dwin = swv(dr, XHALO, axis=2)                          # [B,36,506,12]
        dpl = np.ascontiguousarray(
            dwin[:, :, 0:W:XB, :].transpose(0, 2, 1, 3)).reshape(128, YHALO, XHALO)
        gsel = gw[:, r0:r0 + HS, :, :][..., vals]              # [B,HS,512,D]
        g = np.ascontiguousarray(
            gsel.reshape(B, HS, NXQ, XB, D).transpose(0, 2, 4, 1, 3)).reshape(
            128, D, HS, XB).astype(BF16)
        in_maps.append({"sle": sle, "slo": slo, "g": g, "dpl": dpl, "ident": ident})
    return in_maps, (D, dyv, dxv, pos_d, counts)


def kernel(depth, features, guide_weight, sample_idx):
    depth = np.asarray(depth)
    features = np.asarray(features)
    guide_weight = np.asarray(guide_weight)
    sample_idx = np.asarray(sample_idx)

    in_maps, meta = _prep_inputs(depth, features, guide_weight, sample_idx)
    D, dyv, dxv, pos_d, counts = meta

    key = (tuple(dyv), tuple(dxv), tuple(np.round(pos_d, 10)), tuple(counts))
    nc = _graph_cache.get(key)
    if nc is None:
        nc = _build_graph(D, dyv, dxv, pos_d, counts)
        _graph_cache[key] = nc

    res = run_bass_kernel_spmd(nc, in_maps, core_ids=list(range(NCORES)))

    out = np.empty((B, C, H, W), F32)
    for core in range(NCORES):
        r0 = core * HS
        o = res.results[core]["out"].astype(F32).reshape(B, NXQ, HS, C, XB)
        out[:, :, r0:r0 + HS, :] = o.transpose(0, 3, 2, 1, 4).reshape(B, C, HS, W)
    return out, features


# revision 6
# speedup vs baseline: 1.3479x; 1.1226x over previous
"""Trainium2 Bass kernel for nn_AdaptiveSample (sparse adaptive 5x5 sampling).

Pixel-major rewrite: out[b,c,y,x] = sum_d softmax_d(valid*pos*guide) * f[b,c,y+dy,x+dx]

Sharding: H=256 over 8 cores (32 rows each, halos resolved on host).
Per-core layout: lane = (b, xq) with xq 64 x-blocks of 8 pixels; free = (y, c, xi).
  - softmax weights live in the same lane layout [128, D, y, xi], so the
    per-pixel weight multiplies features via a size-1-axis broadcast over c
    on the DVE directly (no partition broadcast machinery at all).
  - accumulation over the D offsets runs on the TensorEngine as identity
    matmuls accumulating in PSUM (start/stop groups), freeing the DVE of adds.
  - a few product passes run on the Pool engine (gpsimd) to balance DVE.
  - Activation engine does the exps and the 4 PSUM->SBUF evacuations (bf16).
Features are staged as two x-shifted slabs (even/odd dx) so every product AP
starts at an even bf16 element offset.
"""
import os
import sys

for _p in ("/opt/trn_rl_repo",):
    if os.path.isdir(_p) and _p not in sys.path:
        sys.path.append(_p)

import numpy as np
import ml_dtypes

from concourse import bass, mybir
from concourse import tile
from concourse.bass_utils import run_bass_kernel_spmd

BF16 = ml_dtypes.bfloat16
F32 = np.float32

B, C, H, W = 2, 32, 256, 512
KS, PAD, DMAX = 5, 2, 192.0
NCORES = 8
HS = H // NCORES          # 32 rows per core
NXQ = 64                  # x blocks per row
XB = W // NXQ             # 8 pixels per block
XHALO = XB + 2 * PAD      # 12 slab columns per block
YHALO = HS + 2 * PAD      # 36 slab rows
NPIX = HS * XB            # 256 pixel-plane elems per lane

_graph_cache = {}


def _runs(dyv, dxv):
    """Group the D unique offsets into runs of consecutive dx at equal dy.
    Offsets are sorted by (dy, dx) already (np.unique order)."""
    runs = []
    d0 = 0
    D = len(dyv)
    while d0 < D:
        d1 = d0 + 1
        while d1 < D and dyv[d1] == dyv[d0] and dxv[d1] == dxv[d1 - 1] + 1:
            d1 += 1
        runs.append((d0, d1, int(dyv[d0]), int(dxv[d0])))
        d0 = d1
    return runs


def _build_graph(D, dyv, dxv, pos_d, counts, niter=1, pool_units=6):
    """pool_units: how many (d, y-half) product passes run on the Pool engine
    instead of the DVE (Pool is ~3.7x slower per pass but otherwise idle)."""
    nc = bass.Bass(trn_type="TRN2", debug=False, enable_partition_id=False)
    dt_bf = mybir.dt.bfloat16
    dt_f32 = mybir.dt.float32

    sle_p = nc.declare_dram_parameter("sle", [128, YHALO, C, XHALO], dt_bf, isOutput=False)
    slo_p = nc.declare_dram_parameter("slo", [128, YHALO, C, XHALO], dt_bf, isOutput=False)
    g_p = nc.declare_dram_parameter("g", [128, D, HS, XB], dt_bf, isOutput=False)
    dpl_p = nc.declare_dram_parameter("dpl", [128, YHALO, XHALO], dt_bf, isOutput=False)
    id_p = nc.declare_dram_parameter("ident", [128, 128], dt_bf, isOutput=False)
    out_ext = nc.declare_dram_parameter("out", [128, HS, C, XB], dt_bf, isOutput=True)

    MULT = mybir.AluOpType.mult
    ADD = mybir.AluOpType.add
    IS_GT = mybir.AluOpType.is_gt
    IS_LT = mybir.AluOpType.is_lt
    EXP = mybir.ActivationFunctionType.Exp
    COPY = mybir.ActivationFunctionType.Copy

    runs = _runs(dyv, dxv)

    with tile.TileContext(nc) as tc:
        with (
            tc.tile_pool(name="big", bufs=1) as big,
            tc.tile_pool(name="pipe", bufs=1) as pipe,
            tc.tile_pool(name="prod", bufs=6) as prp,
            tc.tile_pool(name="prodp", bufs=4) as prpp,
            tc.tile_pool(name="ob", bufs=2) as obp,
            tc.tile_pool(name="psum", bufs=1, space="PSUM") as psp,
        ):
            g = pipe.tile([128, D, HS, XB], dt_bf, tag="g")
            dpl = pipe.tile([128, YHALO, XHALO], dt_bf, tag="dpl")
            ident = big.tile([128, 128], dt_bf, tag="ident")
            nc.sync.dma_start(out=dpl[:, :, :], in_=dpl_p[:, :, :])
            nc.sync.dma_start(out=g[:, :, :, :], in_=g_p[:, :, :, :])
            nc.sync.dma_start(out=ident[:, :], in_=id_p[:, :])
            sle = big.tile([128, YHALO, C, XHALO], dt_bf, tag="sle")
            slo = big.tile([128, YHALO, C, XHALO], dt_bf, tag="slo")
            # stream the y-halves used first ahead of the rest
            nc.sync.dma_start(out=sle[:, :20, :, :], in_=sle_p[:, :20, :, :])
            nc.sync.dma_start(out=slo[:, :20, :, :], in_=slo_p[:, :20, :, :])
            nc.sync.dma_start(out=sle[:, 20:, :, :], in_=sle_p[:, 20:, :, :])
            nc.sync.dma_start(out=slo[:, 20:, :, :], in_=slo_p[:, 20:, :, :])
            slabs = {0: sle, 1: slo}

            bias_vals = sorted({float(np.log(c_)) for c_ in counts})
            bias_tiles = {}
            for bv in bias_vals:
                bt = pipe.tile([128, 1], dt_f32, tag=f"bias{bv:.4f}")
                nc.vector.memset(bt[:, :], bv)
                bias_tiles[bv] = bt

            for _iter in range(niter):

                # validity plane: v = (dpl > 0) * (dpl < DMAX)  [128, 36, 12]
                t1 = pipe.tile([128, YHALO, XHALO], dt_bf, tag="t1")
                t2 = pipe.tile([128, YHALO, XHALO], dt_bf, tag="t2")
                v = pipe.tile([128, YHALO, XHALO], dt_bf, tag="v")
                nc.vector.tensor_scalar(out=t1[:, :, :], in0=dpl[:, :, :],
                                        scalar1=0.0, scalar2=None, op0=IS_GT)
                nc.vector.tensor_scalar(out=t2[:, :, :], in0=dpl[:, :, :],
                                        scalar1=DMAX, scalar2=None, op0=IS_LT)
                nc.vector.tensor_tensor(v[:, :, :], t1[:, :, :], t2[:, :, :], MULT)

                # vg[d] = g[d] * v[dy_d:dy_d+HS, dx_d:dx_d+XB], runs share one op
                vg = pipe.tile([128, D, HS, XB], dt_bf, tag="vg")
                for (d0, d1, dy, dx0) in runs:
                    n = d1 - d0
                    vb = v[:, dy:dy + HS, dx0:dx0 + XB]
                    vsl = bass.AP(vb.tensor, vb.offset,
                                  [list(vb.ap[0]), [1, n], [XHALO, HS], [1, XB]])
                    nc.vector.tensor_tensor(vg[:, d0:d1, :, :], g[:, d0:d1, :, :],
                                            vsl, MULT)

                # e[d] = count_d * exp(pos_d * vg_d); extra c axis of size 1
                # so slices broadcast over channels later
                e = pipe.tile([128, D, HS, 1, XB], dt_bf, tag="e")
                for d in range(D):
                    bv = float(np.log(counts[d]))
                    nc.scalar.activation(e[:, d, :, 0, :], vg[:, d, :, :], EXP,
                                         bias=bias_tiles[bv][:, :], scale=float(pos_d[d]))

                # denominator tree-fold then reciprocal
                if D > 1:
                    m = D // 2
                    num_f = pipe.tile([128, m, HS, XB], dt_bf, tag="numf")
                    nc.vector.tensor_tensor(num_f[:, :, :, :], e[:, 0:m, :, 0, :],
                                            e[:, m:2 * m, :, 0, :], ADD)
                    lvl = num_f[:, :, :, :]
                    n = m
                    extra = [e[:, 2 * m + i, :, 0, :] for i in range(D - 2 * m)]
                    li = 0
                    while n > 1:
                        h2 = n // 2
                        nt = pipe.tile([128, h2, HS, XB], dt_bf, tag=f"nf{li}")
                        nc.vector.tensor_tensor(nt[:, :, :, :], lvl[:, 0:h2, :, :],
                                                lvl[:, h2:2 * h2, :, :], ADD)
                        if n % 2:
                            extra.append(lvl[:, 2 * h2, :, :])
                        lvl = nt[:, :, :, :]
                        n = h2
                        li += 1
                    cur = lvl[:, 0, :, :]
                    for i, ex in enumerate(extra):
                        dent = pipe.tile([128, HS, XB], dt_bf, tag=f"dx{i}")
                        nc.vector.tensor_tensor(dent[:, :, :], cur, ex, ADD)
                        cur = dent[:, :, :]
                else:
                    cur = e[:, 0, :, 0, :]
                rden_f = pipe.tile([128, HS, XB], dt_f32, tag="rdenf")
                nc.vector.reciprocal(rden_f[:, :, :], cur)
                rden = pipe.tile([128, 1, HS, XB], dt_bf, tag="rden")
                nc.vector.tensor_copy(rden[:, 0, :, :], rden_f[:, :, :])

                # normalized weights w = e * rden  [128, D, HS, 1, XB]
                wq = pipe.tile([128, D, HS, 1, XB], dt_bf, tag="wq")
                rb, _ = bass.broadcast_tensor_aps(rden[:, :, :, :], e[:, :, :, 0, :])
                nc.vector.tensor_tensor(wq[:, :, :, 0, :], e[:, :, :, 0, :], rb, MULT)

                # products + PE identity accumulation, per y-half.
                # pool_units product passes run on the Pool engine; spread the
                # chosen d's away from each other and across halves.
                spread = [d for d in (3, 9, 6, 1, 11, 5, 7, 0, 10, 2, 12, 4, 8) if d < D]
                pool_pairs = set()
                for i in range(min(pool_units, 2 * D)):
                    pool_pairs.add((spread[i % len(spread)], (i + i // len(spread)) % 2))
                for h in range(2):
                    y0 = 16 * h
                    pool_list = [d for d in range(D) if (d, h) in pool_pairs]
                    dve_list = [d for d in range(D) if (d, h) not in pool_pairs]
                    prods = {}
                    # Pool-engine products issued first (slow engine, runs in
                    # parallel with the DVE stream); accumulated last on PE.
                    for d in pool_list + dve_list:
                        dy, dx = int(dyv[d]), int(dxv[d])
                        src = slabs[dx % 2]
                        # slab xi base: even slab xi==dx, odd slab xi==dx-1
                        xs = dx if dx % 2 == 0 else dx - 1
                        f_ap = src[:, dy + y0:dy + y0 + 16, :, xs:xs + XB]
                        w_ap, _ = bass.broadcast_tensor_aps(
                            wq[:, d, y0:y0 + 16, :, :], f_ap)
                        if d in pool_list:
                            pr = prpp.tile([128, 16, C, XB], dt_bf, tag="prp")
                            nc.gpsimd.tensor_tensor(pr[:, :, :, :], f_ap, w_ap, MULT)
                        else:
                            pr = prp.tile([128, 16, C, XB], dt_bf, tag="pr")
                            nc.vector.tensor_tensor(pr[:, :, :, :], f_ap, w_ap, MULT)
                        prods[d] = pr
                    pss = [psp.tile([128, 2048], dt_f32, tag=f"q{q}", name=f"ps_{h}_{q}")
                           for q in range(2)]
                    mm_order = dve_list + pool_list
                    for di, d in enumerate(mm_order):
                        pv = prods[d][:, :, :, :].rearrange("p y c xi -> p (y c xi)")
                        for q in range(2):
                            for ci in range(4):
                                c0 = q * 2048 + ci * 512
                                nc.tensor.matmul(
                                    pss[q][:, ci * 512:(ci + 1) * 512],
                                    lhsT=ident[:, :], rhs=pv[:, c0:c0 + 512],
                                    start=(di == 0), stop=(di == D - 1))
                    for q in range(2):
                        ob = obp.tile([128, 8, C, XB], dt_bf, tag="ob")
                        nc.scalar.activation(
                            ob[:, :, :, :],
                            pss[q][:, :].rearrange("p (y c xi) -> p y c xi",
                                                   y=8, c=C, xi=XB), COPY)
                        nc.sync.dma_start(
                            out=out_ext[:, y0 + 8 * q:y0 + 8 * q + 8, :, :],
                            in_=ob[:, :, :, :])

    _split_excess_waits(nc)
    return nc


def _split_excess_waits(nc, max_waits=1):
    """walrus in this container rejects >1 chained sync-wait per instruction;
    spill extras onto preceding sequencer NOPs."""
    n = 0
    for fn in nc.m.functions:
        for bb in fn.blocks:
            new = []
            for inst in bb.instructions:
                si = inst.sync_info
                w = list(si.on_wait) if si is not None else []
                if len(w) > max_waits:
                    excess = w[max_waits:]
                    si.on_wait = w[:max_waits]
                    for i in range(0, len(excess), max_waits):
                        nop = mybir.InstNoOp(name=nc.get_next_instruction_name(), ins=[], outs=[])
                        nop.engine = inst.engine
                        nsi = nop.sync_info
                        if nsi is None:
                            nop.sync_info = mybir.SyncInfo(on_wait=excess[i:i + max_waits], on_update=[])
                        else:
                            nsi.on_wait = excess[i:i + max_waits]
                        nc.register_instruction(nop)
                        new.append(nop)
                        n += 1
                new.append(inst)
            bb.instructions = new
    return n


def _prep_inputs(depth, features, guide_weight, sample_idx):
    """Shard + lay out the full inputs for the 8 cores. Returns in_maps, meta."""
    si = np.asarray(sample_idx).astype(np.int64)
    vals, counts = np.unique(si, return_counts=True)
    D = len(vals)
    ctr = KS // 2
    px = (si % KS).astype(np.float64)
    py = (si // KS).astype(np.float64)
    Z = np.exp(-0.5 * np.sqrt((px - ctr) ** 2 + (py - ctr) ** 2)).sum()
    pos_d = np.exp(-0.5 * np.sqrt(((vals % KS) - ctr) ** 2 + ((vals // KS) - ctr) ** 2)) / Z
    dyv = (vals // KS).astype(int)          # 0..4 offsets in padded coords
    dxv = (vals % KS).astype(int)

    feats_bf = features.astype(BF16)
    # padded planes: y pad 2 each side; x pad 2 left, 3 right (odd slab shift)
    fpad = np.zeros((B, C, H + 4, W + 5), BF16)
    fpad[:, :, 2:2 + H, 2:2 + W] = feats_bf
    dpad = np.zeros((B, H + 4, W + 5), BF16)
    dpad[:, 2:2 + H, 2:2 + W] = depth.reshape(B, H, W).astype(BF16)

    swv = np.lib.stride_tricks.sliding_window_view  # read-only views
    in_maps = []
    ident = np.eye(128, dtype=BF16)
    gw = np.asarray(guide_weight)
    for core in range(NCORES):
        r0 = core * HS
        fr = fpad[:, :, r0:r0 + YHALO, :]                      # [B,C,36,517]
        win = swv(fr, XHALO, axis=3)                           # [B,C,36,506,12]
        sle = np.ascontiguousarray(
            win[:, :, :, 0:W:XB, :].transpose(0, 3, 2, 1, 4)).reshape(
            128, YHALO, C, XHALO)
        slo = np.ascontiguousarray(
            win[:, :, :, 1:W + 1:XB, :].transpose(0, 3, 2, 1, 4)).reshape(
            128, YHALO, C, XHALO)
        dr = dpad[:, r0:r0 + YHALO, :]                         # [B,36,517]
        dwin = swv(dr, XHALO, axis=2)                          # [B,36,506,12]
        dpl = np.ascontiguousarray(
            dwin[:, :, 0:W:XB, :].transpose(0, 2, 1, 3)).reshape(128, YHALO, XHALO)
        gsel = gw[:, r0:r0 + HS, :, :][..., vals]              # [B,HS,512,D]
        g = np.ascontiguousarray(
            gsel.reshape(B, HS, NXQ, XB, D).transpose(0, 2, 4, 1, 3)).reshape(
            128, D, HS, XB).astype(BF16)
        in_maps.append({"sle": sle, "slo": slo, "g": g, "dpl": dpl, "ident": ident})
    return in_maps, (D, dyv, dxv, pos_d, counts)


def kernel(depth, features, guide_weight, sample_idx):
    depth = np.asarray(depth)
    features = np.asarray(features)
    guide_weight = np.asarray(guide_weight)
    sample_idx = np.asarray(sample_idx)

    in_maps, meta = _prep_inputs(depth, features, guide_weight, sample_idx)
    D, dyv, dxv, pos_d, counts = meta

    key = (tuple(dyv), tuple(dxv), tuple(np.round(pos_d, 10)), tuple(counts))
    nc = _graph_cache.get(key)
    if nc is None:
        nc = _build_graph(D, dyv, dxv, pos_d, counts)
        _graph_cache[key] = nc

    res = run_bass_kernel_spmd(nc, in_maps, core_ids=list(range(NCORES)))

    out = np.empty((B, C, H, W), F32)
    for core in range(NCORES):
        r0 = core * HS
        o = res.results[core]["out"].astype(F32).reshape(B, NXQ, HS, C, XB)
        out[:, :, r0:r0 + HS, :] = o.transpose(0, 3, 2, 1, 4).reshape(B, C, HS, W)
    return out, features
